# revision 7
# baseline (speedup 1.0000x reference)
"""Trainium2 Bass kernel for nn_MicroCoupledSuperNet (GNN message passing supernet).

Strategy (8-core SPMD, dst-node sharding):
  - Each core owns a contiguous range of destination nodes and all edges into them.
  - Per layer, both GCN (sym-normalized, self-loops) and SAGE-mean aggregations are
    computed with ONE matmul per 128-edge tile: gathered-source-rows^T @ E, where
    E carries a [64 gcn cols | 64 sage cols] block of 64 destination nodes,
    accumulated in PSUM.
  - E tiles are BUILT ON-CHIP (not streamed from HBM): the per-edge weights are
    separable, so the gather table is pre-scaled by dinv (y = dinv*x), making the
    gcn half a pure one-hot mask of the in-block dst index (is_equal against a
    resident iota row) and the sage half mask*sqrt(deg_sl[src]).  Per-dst factors
    (dinv_d, 1/deg_d) are applied in the PSUM->SBUF copy (a tensor_tensor mult
    that replaces the plain copy).  Metadata is 4 B/edge, resident, shared by both
    layers -- vs 256 B/edge of streamed E in the naive scheme.
  - Source rows are fetched with dma_gather (int16 indices -> table split in two
    halves).
  - pre-MLP is deferred through the aggregation (A(xW) = (Ax)W), so layer 1 gathers
    straight from the y table; the dense stage fuses conv-mix into 3 matmuls per
    128-node block-pair, followed by a fused LayerNorm-mix + activation-mix chain
    (per-partition tensor_scalar form: hpre = a*z + b2).
  - h1 (dinv-scaled) is exchanged between layers with an AllGather collective.
  - Sum-pool readout is a 0/1 matmul into per-core graph slots; host merges windows
    and adds post_b.
"""

import sys
import math
import dataclasses

import numpy as np

for _p in ("/opt/trn_rl_repo",):
    if _p not in sys.path:
        sys.path.insert(0, _p)

import ml_dtypes  # noqa: E402

BF16 = ml_dtypes.bfloat16

from concourse import bass, bacc, mybir, tile  # noqa: E402
from concourse.bass_utils import run_bass_kernel_spmd  # noqa: E402

P = 128          # SBUF partitions / edge-tile rows
BLK = 64         # destination nodes per aggregation block
H = 128          # hidden dim (== D_IN)
DOUT = 64
SBLK = 8         # aggregation blocks per superblock (scheduling unit)
GSLOTS = 128     # per-core graph slots for pooling
EPS = 1e-5
F32 = mybir.dt.float32
BF = mybir.dt.bfloat16
I16 = mybir.dt.int16
AL = mybir.AluOpType


@dataclasses.dataclass
class Cfg:
    N: int
    E: int
    G: int
    cores: int
    half: int           # gather table split point (int16 index limit)
    nshard: int = 0
    nblk: int = 0
    npair: int = 0
    npad: int = 0
    nsb: int = 0

    def __post_init__(self):
        assert self.N % self.cores == 0
        self.nshard = self.N // self.cores
        self.nblk = math.ceil(self.nshard / BLK)
        if self.nblk % 2:
            self.nblk += 1  # keep whole pairs
        self.npair = self.nblk // 2
        self.npad = self.nblk * BLK
        self.nsb = math.ceil(self.nblk / SBLK)


def _softmax(v):
    v = np.asarray(v, np.float64)
    e = np.exp(v - v.max())
    return e / e.sum()


@dataclasses.dataclass
class Sched:
    """Static (cross-core-uniform) schedule + scalar constants."""
    T: np.ndarray            # [nblk, 2] tiles per (block, half)
    Tc: np.ndarray           # [nblk, 2] gathered idx count per bucket (x16)
    b_idx_off: list          # per block: idx col offset (h0 tiles then h1)
    b_ecol: list             # per block: E-stream col offset (tiles*P)
    idx_cols: int
    ecols: int
    etb_max: int             # max tiles per block (both halves)
    n_tiles: int
    # scalar constants per layer
    wc: np.ndarray           # [L,2]
    wn: np.ndarray           # [L,2]
    wa: np.ndarray           # [L,3]
    gconst: list             # per layer: scalar value of wn0*ln_g row (const)
    have_bias1: bool
    have_bias2: bool
    shard_rows: int          # real rows per shard (nshard)


def _build_schedule(cfg: Cfg, counts: np.ndarray) -> tuple:
    """counts: [cores, nblk, 2] edge counts. Returns tile schedule uniform across cores.
    Streams are block-major: block b's h0 tiles then h1 tiles, contiguous."""
    mx = counts.max(axis=0)
    Tc = (np.ceil(mx / 16) * 16).astype(np.int64)          # gathered idxs (x16)
    T = np.ceil(mx / P).astype(np.int64)                   # matmul tiles
    b_idx_off, b_ecol = [], []
    idx_off = 0
    ecol = 0
    for b in range(cfg.nblk):
        b_idx_off.append(idx_off)
        b_ecol.append(ecol)
        idx_off += int(Tc[b, 0] + Tc[b, 1]) // 16
        ecol += int(T[b, 0] + T[b, 1]) * P
    etb_max = int((T[:, 0] + T[:, 1]).max())
    return T, Tc, b_idx_off, b_ecol, idx_off, ecol, etb_max


def host_prep(inputs: dict, cfg: Cfg):
    """Numpy preprocessing: edge bucketing/tiling, metadata stream, index stream,
    combined weight matrices. Returns (sched, per-core in_maps data, combine info)."""
    x = np.asarray(inputs["x"], np.float32)
    ei = np.asarray(inputs["edge_index"])
    batch = np.asarray(inputs["batch"]).astype(np.int64)
    src = ei[0].astype(np.int64)
    dst = ei[1].astype(np.int64)
    N, E, G_N, C = cfg.N, cfg.E, cfg.G, cfg.cores
    ns = cfg.nshard

    deg_sl = np.bincount(dst, minlength=N).astype(np.float64) + 1.0  # with self loop
    dinv = 1.0 / np.sqrt(deg_sl)
    sqrtdeg = np.sqrt(deg_sl)             # 1/dinv, per-source sage row factor
    degn = np.maximum(np.bincount(dst, minlength=N), 1).astype(np.float64)

    # ---- per-core edge lists (with self-loop pseudo-edges) ----
    per_core = []
    counts = np.zeros((C, cfg.nblk, 2), np.int64)
    for c in range(C):
        lo, hi = c * ns, (c + 1) * ns
        m = (dst >= lo) & (dst < hi)
        es, ed = src[m], dst[m]
        dd = np.arange(lo, hi, dtype=np.int64)
        asrc = np.concatenate([es, dd])
        adst = np.concatenate([ed, dd])
        # per-edge row factor for sage half (0 for self-loop pseudo-edges)
        rf = np.concatenate([sqrtdeg[es], np.zeros(ns)])
        dloc = adst - lo
        blk = dloc // BLK
        din = dloc % BLK
        hf = (asrc >= cfg.half).astype(np.int64)
        order = np.lexsort((hf, blk))
        asrc, rf, blk, din, hf = (a[order] for a in (asrc, rf, blk, din, hf))
        for b in range(cfg.nblk):
            mb = blk == b
            counts[c, b, 0] = int((mb & (hf == 0)).sum())
            counts[c, b, 1] = int((mb & (hf == 1)).sum())
        per_core.append((asrc, rf, blk, din, hf))

    T, Tc, b_idx_off, b_ecol, idx_cols, ecols, etb_max = _build_schedule(cfg, counts)
    n_tiles_total = int(T.sum())

    # ---- pack per-core index + metadata streams ----
    data = []
    for c in range(C):
        asrc, rf, blk, din, hf = per_core[c]
        tile_base = {}
        idx_base = {}
        tix = 0
        cix = 0
        for b in range(cfg.nblk):
            for hh in (0, 1):
                tile_base[(b, hh)] = tix
                idx_base[(b, hh)] = cix
                tix += int(T[b, hh])
                cix += int(Tc[b, hh])
        assert tix == n_tiles_total
        idx_total = cix
        # scatter edges into tile slots
        key = blk * 2 + hf
        order = np.argsort(key, kind="stable")
        asrc, rf, blk, din, hf = (a[order] for a in (asrc, rf, blk, din, hf))
        pos = np.zeros(len(asrc), np.int64)
        start = 0
        for b in range(cfg.nblk):
            for hh in (0, 1):
                nbh = counts[c, b, hh]
                pos[start:start + nbh] = np.arange(nbh)
                start += nbh
        tno = np.array([tile_base[(int(b), int(h))] for b, h in zip(blk, hf)]) + pos // P
        prow = pos % P
        idxval = np.where(hf == 0, asrc, asrc - cfg.half)
        # metadata: per tile 2 columns [din | rowfac]; pad rows din=127, rf=0
        mdfull = np.zeros((n_tiles_total, P, 2), np.float32)
        mdfull[:, :, 0] = 127.0
        mdfull[tno, prow, 0] = din
        mdfull[tno, prow, 1] = rf
        md = np.ascontiguousarray(
            mdfull.transpose(1, 0, 2).reshape(P, n_tiles_total * 2)).astype(np.float32)
        # idx stream: per-bucket Tc-sized ranges (gathers run at 16-idx
        # granularity; pads use index 0 and din=127 -> zero E columns)
        ipos = np.array([idx_base[(int(b), int(h))] for b, h in zip(blk, hf)]) + pos
        flat = np.zeros(idx_total, np.int64)
        flat[ipos] = idxval
        wrapped = flat.reshape(-1, 16).T  # [16, total/16]
        idx16 = np.tile(wrapped, (8, 1)).astype(np.int16)  # [128, cols]
        assert idx16.shape[1] == idx_cols
        data.append({"md": md, "idx": idx16})

    # ---- pooling ----
    g_lo = []
    for c in range(C):
        lo = int(batch[c * ns])
        hi = int(batch[(c + 1) * ns - 1])
        span = hi - lo + 1
        assert span <= GSLOTS, f"graph span {span} exceeds {GSLOTS}"
        g_lo.append(lo)
        ep = np.zeros((cfg.npad, GSLOTS), np.float32)
        rows = np.arange(ns)
        ep[rows, batch[c * ns:(c + 1) * ns] - lo] = 1.0
        epm = np.ascontiguousarray(
            ep.reshape(cfg.npair, P, GSLOTS).transpose(1, 0, 2)
            .reshape(P, cfg.npair * GSLOTS)).astype(BF16)
        data[c]["epool"] = epm

    # ---- weights / constants ----
    pre_w = np.asarray(inputs["pre_w"], np.float64)
    pre_b = np.asarray(inputs["pre_b"], np.float64)
    post_w = np.asarray(inputs["post_w"], np.float64)
    post_b = np.asarray(inputs["post_b"], np.float64)
    gcn_w = np.asarray(inputs["gcn_w"], np.float64)
    gcn_b = np.asarray(inputs["gcn_b"], np.float64)
    sage_ws = np.asarray(inputs["sage_ws"], np.float64)
    sage_wn = np.asarray(inputs["sage_wn"], np.float64)
    ln_g = np.asarray(inputs["ln_g"], np.float64)
    ln_b = np.asarray(inputs["ln_b"], np.float64)
    a_conv = np.asarray(inputs["a_conv"], np.float64)
    a_norm = np.asarray(inputs["a_norm"], np.float64)
    a_act = np.asarray(inputs["a_act"], np.float64)

    wc = np.stack([_softmax(a_conv[l]) for l in range(2)])
    wn = np.stack([_softmax(a_norm[l]) for l in range(2)])
    wa = np.stack([_softmax(a_act[l]) for l in range(2)])

    # ln_g rows must be constant for the fused a*z+b2 LayerNorm path
    gconst = []
    for l in range(2):
        row = wn[l, 0] * ln_g[l]
        assert np.all(row == row[0]), "non-constant ln_g unsupported by fast path"
        assert np.abs(wn[l, 0] * ln_b[l]).max() == 0, "ln_b must be zero"
        gconst.append(float(row[0]))

    Vg1 = pre_w @ (wc[0, 0] * gcn_w[0])
    VI1 = pre_w @ (wc[0, 1] * sage_ws[0])
    Vs1 = pre_w @ (wc[0, 1] * sage_wn[0])
    Vg2 = wc[1, 0] * gcn_w[1]
    VI2 = wc[1, 1] * sage_ws[1]
    Vs2 = wc[1, 1] * sage_wn[1]
    vm = np.stack([Vg1, VI1, Vs1, Vg2, VI2, Vs2]).astype(BF16)

    qg = wc[0, 0] * (pre_b @ gcn_w[0])
    qs = wc[0, 1] * (pre_b @ sage_wn[0])
    qc = wc[0, 0] * gcn_b[0] + wc[0, 1] * (pre_b @ sage_ws[0])
    bc2 = wc[1, 0] * gcn_b[1]
    qv = np.stack([qg, qs, qc, bc2]).astype(BF16)
    have_bias1 = bool(np.abs(qv[:3]).max() > 0)
    have_bias2 = bool(np.abs(bc2).max() > 0)

    # rs vectors (per-core, padded) for bias folding
    rs_gcn_full = np.zeros(N)
    np.add.at(rs_gcn_full, dst, dinv[src])
    rs_gcn_full = dinv * rs_gcn_full + dinv ** 2
    rs_sage_full = (np.bincount(dst, minlength=N) > 0).astype(np.float64)
    for c in range(C):
        r = np.zeros((3, cfg.npad), np.float32)
        r[0, :ns] = rs_gcn_full[c * ns:(c + 1) * ns]
        r[1, :ns] = rs_sage_full[c * ns:(c + 1) * ns]
        r[2, :] = 1.0
        data[c]["rsv"] = r.astype(BF16)

    # gather table: y = dinv * x (per-source gcn factor folded in)
    yb = (dinv[:, None] * x).astype(BF16)
    iota64 = np.tile(np.arange(BLK, dtype=np.float32), (P, 1)).astype(BF16)
    for c in range(C):
        lo = c * ns
        xs = np.zeros((cfg.npad, H), np.float32)
        xs[:ns] = x[lo:lo + ns]
        data[c]["xst"] = np.ascontiguousarray(xs.T).astype(BF16)
        data[c]["xb"] = yb
        data[c]["vm"] = vm
        data[c]["qv"] = qv
        data[c]["pw"] = post_w.astype(BF16)
        data[c]["ident"] = np.eye(P, dtype=np.float32).astype(BF16)
        data[c]["iota"] = iota64
        # per-dst factors, replicated across partitions
        dv = np.ones(cfg.npad, np.float64)
        dv[:ns] = dinv[lo:lo + ns]
        idg = np.ones(cfg.npad, np.float64)
        idg[:ns] = 1.0 / degn[lo:lo + ns]
        data[c]["dinvrep"] = np.tile(dv.astype(np.float32), (P, 1)).astype(BF16)
        data[c]["idegrep"] = np.tile(idg.astype(np.float32), (P, 1)).astype(BF16)
        # dinv node-major columns (for scaling h1 before the collective)
        dnm = np.ones((P, cfg.npair), np.float64)
        for pr in range(cfg.npair):
            seg = dv[pr * P:(pr + 1) * P]
            dnm[:len(seg), pr] = seg
        data[c]["dinvnm"] = dnm.astype(np.float32)

    sched = Sched(T=T, Tc=Tc, b_idx_off=b_idx_off, b_ecol=b_ecol,
                  idx_cols=idx_cols, ecols=ecols, etb_max=etb_max,
                  n_tiles=n_tiles_total,
                  wc=wc, wn=wn, wa=wa, gconst=gconst,
                  have_bias1=have_bias1, have_bias2=have_bias2,
                  shard_rows=ns)
    combine = {"g_lo": g_lo, "post_b": post_b}
    return sched, data, combine


def build_program(cfg: Cfg, sched: Sched):
    nc = bacc.Bacc("TRN2", target_bir_lowering=False, debug=False,
                   enable_asserts=False, num_devices=cfg.cores,
                   num_swdge_queues=4)

    xb_d = nc.dram_tensor("xb", [cfg.N, H], BF, kind="ExternalInput")
    xst_d = nc.dram_tensor("xst", [H, cfg.npad], BF, kind="ExternalInput")
    idx_d = nc.dram_tensor("idx", [P, sched.idx_cols], I16, kind="ExternalInput")
    md_d = nc.dram_tensor("md", [P, sched.n_tiles * 2], F32, kind="ExternalInput")
    epool_d = nc.dram_tensor("epool", [P, cfg.npair * GSLOTS], BF, kind="ExternalInput")
    vm_d = nc.dram_tensor("vm", [6, P, H], BF, kind="ExternalInput")
    qv_d = nc.dram_tensor("qv", [4, H], BF, kind="ExternalInput")
    rsv_d = nc.dram_tensor("rsv", [3, cfg.npad], BF, kind="ExternalInput")
    pw_d = nc.dram_tensor("pw", [H, DOUT], BF, kind="ExternalInput")
    ident_d = nc.dram_tensor("ident", [P, P], BF, kind="ExternalInput")
    iota_d = nc.dram_tensor("iota", [P, BLK], BF, kind="ExternalInput")
    dinvrep_d = nc.dram_tensor("dinvrep", [P, cfg.npad], BF, kind="ExternalInput")
    idegrep_d = nc.dram_tensor("idegrep", [P, cfg.npad], BF, kind="ExternalInput")
    dinvnm_d = nc.dram_tensor("dinvnm", [P, cfg.npair], F32, kind="ExternalInput")
    out_d = nc.dram_tensor("out_part", [GSLOTS, DOUT], F32, kind="ExternalOutput")

    h1s_d = nc.dram_tensor("h1s", [cfg.nshard, H], BF)           # shard (collective in)
    h1f_d = nc.dram_tensor("h1f", [cfg.N, H], BF, addr_space="Shared")  # collective out

    ns = cfg.nshard

    with tile.TileContext(nc) as tc:
        with (
            tc.tile_pool(name="const", bufs=1) as cpool,
            tc.tile_pool(name="eb", bufs=4) as ebpool,
            tc.tile_pool(name="pairs", bufs=12) as prpool,
            tc.tile_pool(name="lnt", bufs=3) as lnpool,
            tc.tile_pool(name="stat", bufs=4) as stpool,
            tc.tile_pool(name="small", bufs=4) as smpool,
            tc.tile_pool(name="ps_agg", bufs=2, space="PSUM") as ps_agg,
            tc.tile_pool(name="ps_dense", bufs=4, space="PSUM") as ps_dense,
            tc.tile_pool(name="ps_tr", bufs=1, space="PSUM") as ps_tr,
            tc.tile_pool(name="ps_pool", bufs=1, space="PSUM") as ps_pool,
        ):
            # ---------- resident constants ----------
            idx_t = cpool.tile([P, sched.idx_cols], I16)
            nc.sync.dma_start(out=idx_t[:], in_=idx_d.ap())
            md_t = cpool.tile([P, sched.n_tiles * 2], F32)
            nc.sync.dma_start(out=md_t[:], in_=md_d.ap())
            epool_t = cpool.tile([P, cfg.npair * GSLOTS], BF)
            nc.sync.dma_start(out=epool_t[:], in_=epool_d.ap())
            vm_t = []
            for i in range(6):
                t = cpool.tile([P, H], BF, tag=f"vm{i}")
                nc.sync.dma_start(out=t[:], in_=vm_d.ap()[i])
                vm_t.append(t)
            qv_t = []
            for i in range(4):
                t = cpool.tile([1, H], BF, tag=f"qv{i}")
                nc.sync.dma_start(out=t[:], in_=qv_d.ap()[i:i + 1, :])
                qv_t.append(t)
            rsv_t = []
            for i in range(3):
                t = cpool.tile([1, cfg.npad], BF, tag=f"rsv{i}")
                nc.sync.dma_start(out=t[:], in_=rsv_d.ap()[i:i + 1, :])
                rsv_t.append(t)
            pw_t = cpool.tile([H, DOUT], BF)
            nc.sync.dma_start(out=pw_t[:], in_=pw_d.ap())
            ident_t = cpool.tile([P, P], BF)
            nc.sync.dma_start(out=ident_t[:], in_=ident_d.ap())
            iota_t = cpool.tile([P, BLK], BF)
            nc.sync.dma_start(out=iota_t[:], in_=iota_d.ap())
            dinvrep_t = cpool.tile([P, cfg.npad], BF)
            nc.sync.dma_start(out=dinvrep_t[:], in_=dinvrep_d.ap())
            idegrep_t = cpool.tile([P, cfg.npad], BF)
            nc.sync.dma_start(out=idegrep_t[:], in_=idegrep_d.ap())
            dinvnm_t = cpool.tile([P, cfg.npair], F32)
            nc.sync.dma_start(out=dinvnm_t[:], in_=dinvnm_d.ap())
            xst_t = cpool.tile([P, cfg.npad], BF)      # feature-major x (own shard)
            nc.sync.dma_start(out=xst_t[:], in_=xst_d.ap())
            h1T_t = cpool.tile([P, cfg.npad], BF)      # feature-major h1 (own shard)
            h1loc_t = cpool.tile([P, cfg.npair * H], BF)  # node-major h1 (own shard)
            # explicit gather-buffer ring: deterministic slots, zeroed once so
            # rows never gathered stay finite (their E columns are zero)
            gb_ring = []
            for i in range(4):
                t = cpool.tile([P, max(sched.etb_max, 1) * P], BF, tag=f"gbr{i}")
                nc.vector.memset(t[:], 0)
                gb_ring.append(t)

            pool_psum = ps_pool.tile([GSLOTS, H], F32)

            self_incr = [0]  # round-robin counter for SWDGE queues
            ebuild_incr = [0]  # round-robin E-build engine

            def run_layer(l):
                wn1 = float(sched.wn[l, 1])
                gc = sched.gconst[l]
                ra = float(sched.wa[l, 0] + sched.wa[l, 2])
                ta = float(sched.wa[l, 1])
                ea = float(sched.wa[l, 2])
                bias_mm = sched.have_bias1 if l == 0 else sched.have_bias2
                table = xb_d.ap() if l == 0 else h1f_d.ap()
                tab_lo = table[0:cfg.half]
                tab_hi = table[cfg.half:cfg.N]

                for sb in range(cfg.nsb):
                    b0, b1 = sb * SBLK, min((sb + 1) * SBLK, cfg.nblk)
                    npr = (b1 - b0) // 2
                    pr0 = b0 // 2

                    gp = [None] * npr
                    sp = [None] * npr
                    for b in range(b0, b1):
                        nt0 = int(sched.T[b, 0])
                        nt1 = int(sched.T[b, 1])
                        ntb = nt0 + nt1
                        iob = sched.b_idx_off[b]
                        gt0 = sched.b_ecol[b] // P   # global tile index base
                        gb = gb_ring[b % 4]
                        eb = ebpool.tile([P, sched.etb_max * P], BF, tag="ebb",
                                         name=f"eb_{l}_{b}")
                        # dma_gather dies above 1024 indices/instruction
                        # (ucode index-buffer limit) -> one gather per
                        # (block, half) bucket. Round-robin the 4 SWDGE queues.
                        nc0 = int(sched.Tc[b, 0])
                        nc1 = int(sched.Tc[b, 1])
                        for hh, t0, tn, cn, co in ((0, 0, nt0, nc0, 0),
                                                   (1, nt0, nt1, nc1, nc0)):
                            if cn == 0:
                                continue
                            assert cn <= 1024, "bucket exceeds gather limit"
                            tabn = tab_lo if hh == 0 else tab_hi
                            nc.gpsimd.dma_gather(
                                out_ap=gb[:, t0 * P:(t0 + tn) * P]
                                .rearrange("p (t c) -> p t c", c=P),
                                in_ap=tabn,
                                idxs_ap=idx_t[:, iob + co // 16:
                                              iob + (co + cn) // 16],
                                num_idxs=cn, num_idxs_reg=cn, elem_size=H,
                                queue_num=self_incr[0] % 4)
                            self_incr[0] += 1
                        # build E tiles on-chip: gcn half = onehot(din),
                        # sage half = onehot(din) * sqrtdeg_src
                        for k in range(ntb):
                            gt = gt0 + k
                            eng = (nc.vector if ebuild_incr[0] % 2 == 0
                                   else nc.gpsimd)
                            ebuild_incr[0] += 1
                            eng.tensor_scalar(
                                out=eb[:, k * P:k * P + BLK], in0=iota_t[:],
                                scalar1=md_t[:, 2 * gt:2 * gt + 1],
                                scalar2=None, op0=AL.is_equal)
                            eng.tensor_scalar(
                                out=eb[:, k * P + BLK:(k + 1) * P], in0=iota_t[:],
                                scalar1=md_t[:, 2 * gt:2 * gt + 1],
                                scalar2=md_t[:, 2 * gt + 1:2 * gt + 2],
                                op0=AL.is_equal, op1=AL.mult)

                        ps = ps_agg.tile([P, P], F32, tag="agg")
                        for k in range(ntb):
                            nc.tensor.matmul(
                                ps[:],
                                lhsT=gb[:, k * P:(k + 1) * P],
                                rhs=eb[:, k * P:(k + 1) * P],
                                start=(k == 0), stop=(k == ntb - 1))
                        prl = (b - b0) // 2
                        side = b % 2
                        if side == 0:
                            gp[prl] = prpool.tile([P, P], BF, tag="gp", name=f"gp_{l}_{b}")
                            sp[prl] = prpool.tile([P, P], BF, tag="sp", name=f"sp_{l}_{b}")
                        # PSUM->SBUF copy fused with the per-dst factors
                        nc.vector.tensor_tensor(
                            out=gp[prl][:, side * BLK:(side + 1) * BLK],
                            in0=ps[:, 0:BLK],
                            in1=dinvrep_t[:, b * BLK:(b + 1) * BLK],
                            op=AL.mult)
                        nc.vector.tensor_tensor(
                            out=sp[prl][:, side * BLK:(side + 1) * BLK],
                            in0=ps[:, BLK:2 * BLK],
                            in1=idegrep_t[:, b * BLK:(b + 1) * BLK],
                            op=AL.mult)

                    # ---- dense stage + LayerNorm stats per pair ----
                    F = npr * H
                    pos = []
                    s1 = stpool.tile([P, max(npr, 1)], F32, tag="s1")
                    s2 = stpool.tile([P, max(npr, 1)], F32, tag="s2")
                    junk = lnpool.tile([P, H], F32, tag="junk")
                    for prl in range(npr):
                        pr = pr0 + prl
                        hsrc = xst_t if l == 0 else h1T_t
                        hT_ap = hsrc[:, pr * P:(pr + 1) * P]
                        po = ps_dense.tile([P, H], F32, tag="dense",
                                           name=f"po_{l}_{pr}")
                        nc.tensor.matmul(po[:], lhsT=gp[prl][:], rhs=vm_t[3 * l + 0][:],
                                         start=True, stop=False)
                        nc.tensor.matmul(po[:], lhsT=hT_ap, rhs=vm_t[3 * l + 1][:],
                                         start=False, stop=False)
                        nc.tensor.matmul(po[:], lhsT=sp[prl][:], rhs=vm_t[3 * l + 2][:],
                                         start=False, stop=not bias_mm)
                        if bias_mm:
                            if l == 0:
                                nc.tensor.matmul(po[:], lhsT=rsv_t[0][:, pr * P:(pr + 1) * P],
                                                 rhs=qv_t[0][:], start=False, stop=False)
                                nc.tensor.matmul(po[:], lhsT=rsv_t[1][:, pr * P:(pr + 1) * P],
                                                 rhs=qv_t[1][:], start=False, stop=False)
                                nc.tensor.matmul(po[:], lhsT=rsv_t[2][:, pr * P:(pr + 1) * P],
                                                 rhs=qv_t[2][:], start=False, stop=True)
                            else:
                                nc.tensor.matmul(po[:], lhsT=rsv_t[2][:, pr * P:(pr + 1) * P],
                                                 rhs=qv_t[3][:], start=False, stop=True)
                        pos.append(po)
                        nc.vector.tensor_reduce(out=s1[:, prl:prl + 1], in_=po[:],
                                                axis=mybir.AxisListType.X,
                                                op=AL.add)
                        nc.scalar.activation(
                            out=junk[:], in_=po[:],
                            func=mybir.ActivationFunctionType.Square,
                            accum_out=s2[:, prl:prl + 1])

                    # ---- batched tiny LN math: a = gc*rstd + wn1, b2 = -gc*mu*rstd
                    mu = stpool.tile([P, max(npr, 1)], F32, tag="mu")
                    nc.vector.tensor_scalar(out=mu[:, :npr], in0=s1[:, :npr],
                                            scalar1=1.0 / H, scalar2=None,
                                            op0=AL.mult)
                    m2 = stpool.tile([P, max(npr, 1)], F32, tag="m2")
                    nc.vector.tensor_tensor(out=m2[:, :npr], in0=mu[:, :npr],
                                            in1=mu[:, :npr], op=AL.mult)
                    ve = stpool.tile([P, max(npr, 1)], F32, tag="ve")
                    nc.vector.tensor_scalar(out=ve[:, :npr], in0=m2[:, :npr],
                                            scalar1=-1.0, scalar2=EPS,
                                            op0=AL.mult, op1=AL.add)
                    nc.vector.tensor_scalar(out=s2[:, :npr], in0=s2[:, :npr],
                                            scalar1=1.0 / H, scalar2=None,
                                            op0=AL.mult)
                    nc.vector.tensor_tensor(out=ve[:, :npr], in0=ve[:, :npr],
                                            in1=s2[:, :npr], op=AL.add)
                    sd = stpool.tile([P, max(npr, 1)], F32, tag="sd")
                    nc.scalar.activation(out=sd[:, :npr], in_=ve[:, :npr],
                                         func=mybir.ActivationFunctionType.Sqrt)
                    ri = stpool.tile([P, max(npr, 1)], F32, tag="ri")
                    nc.vector.reciprocal(out=ri[:, :npr], in_=sd[:, :npr])
                    av = stpool.tile([P, max(npr, 1)], F32, tag="av")
                    nc.vector.tensor_scalar(out=av[:, :npr], in0=ri[:, :npr],
                                            scalar1=gc, scalar2=wn1,
                                            op0=AL.mult, op1=AL.add)
                    b2 = stpool.tile([P, max(npr, 1)], F32, tag="b2")
                    nc.vector.tensor_tensor(out=b2[:, :npr], in0=mu[:, :npr],
                                            in1=ri[:, :npr], op=AL.mult)
                    nc.vector.tensor_scalar(out=b2[:, :npr], in0=b2[:, :npr],
                                            scalar1=-gc, scalar2=None,
                                            op0=AL.mult)

                    # ---- hpre = a*z + b2 per pair (reads dense PSUM) ----
                    hp = lnpool.tile([P, max(npr, 1) * H], BF, tag="hp")
                    for prl in range(npr):
                        nc.vector.tensor_scalar(
                            out=hp[:, prl * H:(prl + 1) * H], in0=pos[prl][:],
                            scalar1=av[:, prl:prl + 1], scalar2=b2[:, prl:prl + 1],
                            op0=AL.mult, op1=AL.add)

                    # ---- activation mix (batched, bf16):
                    # out = ra*relu(h) + ta*tanh(h) + ea*exp(min(h,0)) - ea
                    th = lnpool.tile([P, max(npr, 1) * H], BF, tag="th")
                    nc.scalar.activation(out=th[:, :F], in_=hp[:, :F],
                                         func=mybir.ActivationFunctionType.Tanh)
                    mm = lnpool.tile([P, max(npr, 1) * H], BF, tag="mm")
                    nc.vector.tensor_scalar(out=mm[:, :F], in0=hp[:, :F],
                                            scalar1=0.0, scalar2=None, op0=AL.min)
                    nc.scalar.activation(out=mm[:, :F], in_=mm[:, :F],
                                         func=mybir.ActivationFunctionType.Exp)
                    nc.scalar.activation(out=hp[:, :F], in_=hp[:, :F],
                                         func=mybir.ActivationFunctionType.Relu,
                                         scale=ra)
                    nc.vector.tensor_scalar(out=mm[:, :F], in0=mm[:, :F],
                                            scalar1=ea, scalar2=-ea,
                                            op0=AL.mult, op1=AL.add)
                    nc.vector.scalar_tensor_tensor(out=hp[:, :F], in0=th[:, :F],
                                                   scalar=ta, in1=hp[:, :F],
                                                   op0=AL.mult, op1=AL.add)
                    if l == 0:
                        hdst = h1loc_t[:, pr0 * H:pr0 * H + F]
                    else:
                        h2sb = lnpool.tile([P, max(npr, 1) * H], BF, tag="h2")
                        hdst = h2sb[:, :F]
                    nc.vector.tensor_tensor(out=hdst, in0=hp[:, :F], in1=mm[:, :F],
                                            op=AL.add)

                    if l == 0:
                        hsc = lnpool.tile([P, max(npr, 1) * H], BF, tag="hsc")
                        for prl in range(npr):
                            pr = pr0 + prl
                            # dinv-scaled copy for the gather table
                            nc.vector.tensor_scalar(
                                out=hsc[:, prl * H:(prl + 1) * H],
                                in0=h1loc_t[:, pr * H:(pr + 1) * H],
                                scalar1=dinvnm_t[:, pr:pr + 1], scalar2=None,
                                op0=AL.mult)
                            rows = min(P, ns - pr * P)
                            if rows > 0:
                                nc.sync.dma_start(
                                    out=h1s_d.ap()[pr * P:pr * P + rows, :],
                                    in_=hsc[0:rows, prl * H:(prl + 1) * H])
                            pt = ps_tr.tile([P, P], BF, tag="tr")
                            nc.tensor.transpose(out=pt[:],
                                                in_=h1loc_t[:, pr * H:(pr + 1) * H],
                                                identity=ident_t[:])
                            nc.vector.tensor_copy(out=h1T_t[:, pr * P:(pr + 1) * P],
                                                  in_=pt[:])
                    else:
                        skip = h2sb
                        nc.vector.tensor_tensor(out=skip[:, :F],
                                                in0=h1loc_t[:, pr0 * H:pr0 * H + F],
                                                in1=hdst, op=AL.add)
                        for prl in range(npr):
                            pr = pr0 + prl
                            nc.tensor.matmul(
                                pool_psum[:],
                                lhsT=epool_t[:, pr * GSLOTS:(pr + 1) * GSLOTS],
                                rhs=skip[:, prl * H:(prl + 1) * H],
                                start=(pr == 0), stop=(pr == cfg.npair - 1))

            run_layer(0)
            nc.gpsimd.collective_compute(
                "AllGather", mybir.AluOpType.bypass,
                replica_groups=[list(range(cfg.cores))],
                ins=[h1s_d.ap()], outs=[h1f_d.ap()])
            run_layer(1)

            # ---------- readout: pooled @ post_w ----------
            poolc = smpool.tile([GSLOTS, H], BF, tag="poolc")
            nc.vector.tensor_copy(out=poolc[:], in_=pool_psum[:])
            pt = ps_tr.tile([P, GSLOTS], BF, tag="tr")
            nc.tensor.transpose(out=pt[:], in_=poolc[:], identity=ident_t[:])
            ptc = smpool.tile([P, GSLOTS], BF, tag="ptc")
            nc.vector.tensor_copy(out=ptc[:], in_=pt[:])
            ops = ps_dense.tile([GSLOTS, DOUT], F32, tag="dense")
            nc.tensor.matmul(ops[:], lhsT=ptc[:], rhs=pw_t[:], start=True, stop=True)
            outc = smpool.tile([GSLOTS, DOUT], F32, tag="outc")
            nc.vector.tensor_copy(out=outc[:], in_=ops[:])
            nc.sync.dma_start(out=out_d.ap(), in_=outc[:])

    nc.compile()
    return nc


def _kernel_impl(inputs: dict, cfg: Cfg = None, trace: bool = False):
    if cfg is None:
        cfg = Cfg(N=50000, E=640000, G=500, cores=8, half=32768)
    sched, data, combine = host_prep(inputs, cfg)
    nc = build_program(cfg, sched)
    in_maps = [data[c] for c in range(cfg.cores)]
    res = run_bass_kernel_spmd(nc, in_maps, core_ids=list(range(cfg.cores)),
                               trace=trace)
    out = np.zeros((cfg.G, DOUT), np.float64)
    for c in range(cfg.cores):
        part = np.asarray(res.results[c]["out_part"], np.float64)
        lo = combine["g_lo"][c]
        hi = min(lo + GSLOTS, cfg.G)
        out[lo:hi] += part[:hi - lo]
    out += combine["post_b"]
    return out.astype(np.float32), res


def kernel(**inputs) -> np.ndarray:
    out, _ = _kernel_impl(inputs)
    return out
